# revision 28
# baseline (speedup 1.0000x reference)
"""Trainium2 Bass kernel for nn_Cross_Attention (B=8, N=2048, D=768).

Math (per batch b):
    A   = softmax(t, axis=-1) = E/R     (E = exp(t), R = rowsum)
    Q   = softmax(t, axis=0)  = E/S     (S = colsum)
    attn = (x @ A^T) @ Q = x @ KQ,   KQ[d,d'] = (sum_m E[m,d] E[m,d']/R[m]) / S[d']
    out = x @ Msum + fb + x
    Msum = f*(KQ_1 @ W1^T + KQ_2 @ W2^T),  fb = f*(b1 + b2),  f = sigmoid(w)

All heavy matmuls run in fp8e4 with MatmulPerfMode.DoubleRow (2 k-tiles
per instruction, 2x PE rate).  The 1/R row normalization is applied to
the *stationary* operand only (sc = E * C1/R), so the KQ Gram matmuls
for a token pair depend only on that pair's exp + rowsum.

Schedule: x2 and x3 stream CONCURRENTLY (x2 2-tile pairs on the sync
HWDGE queue, x3 4-tile chunks on the scalar HWDGE queue) and their exp /
colsum / KQ-dp0 work interleaves per pair.  The remaining KQ rows
(dp1-5) don't fit PSUM alongside both colsum accumulators, so they run
as pure-PE bursts right after each attention's stream ends (g and sc
stay resident).  KQ_raw is symmetric: only upper blocks are computed;
lower blocks are PE-transposed and rescaled (S[lo]/S[hi]) on gpsimd.

Scale plan (fp8 ranges):
    g   = E                    in [0.008, 250]
    sc  = E * C1/R, C1=1024    <= ~280
    kqt = (C1*KQraw) * (CKQ/C1)/S = CKQ*KQ, CKQ=256   ~0.3
    wts = 32*f*W^T             ~ +-0.9  (cast on host)
    msum = (CKQ*32*Msum)/8 = 1024*Msum ~ +-2.7
    xti = 8*x^T  (scale rides the psum->sbuf copy)
    y_ps = xti @ msum + (ones/128)^T @ (8192*fb) = 8192*(x@Msum + fb)
    out = y_ps/8192 + x   (exact f32 residual), written bf16 via gpsimd
"""

import numpy as np
import ml_dtypes

import concourse.bass as bass
import concourse.tile as tile
from concourse import bacc
from concourse import mybir
from concourse.bass_utils import run_bass_kernel_spmd

F32 = mybir.dt.float32
BF16 = mybir.dt.bfloat16
FP8 = mybir.dt.float8e4
DR = mybir.MatmulPerfMode.DoubleRow
MUL = mybir.AluOpType.mult

B = 8
P = 128
D = 768
DT = D // P  # 6 feature tiles
C1 = 1024.0
CKQ = 256.0
CW = 32.0
CM_DIV = 8.0          # msum = msum_ps / 8
CY = 8.0              # xti = 8*x^T
Y_SCALE = 1.0 / (CKQ * CW / CM_DIV * CY)  # 1/8192

# moving-dim chunks: each must stay inside one PSUM bank (512 f32)
CHUNKS = ((0, 512), (512, 256))

# ---- PSUM map (f32 offsets; banks are 512 f32) ----
# stream phase: s_ps[t0]@3072, s_ps[t1]@2048, KQ dp0 row t0@0, t1@1024
# post bursts (dp1-5, per t, reusing freed regions): see POST_PLAN
# B: m_ps @0/1024;  C: y_ps @0/1024, x^T batches @2048/3072
# transpose slots @3840/3968 always
S_OFF = (3072, 2048)
DP0_OFF = (0, 1024)
TP_OFF = (3840, 3968)
# (col_off, width, psum_off) per dp for the post pass
POST_PLAN = {
    1: ((128, 384, 0), (512, 256, 512)),
    2: ((256, 256, 768), (512, 256, 1792)),
    3: ((256, 256, 2816), (512, 256, 3072)),
    4: ((512, 256, 3328),),
    5: ((512, 256, 3584),),
}
SYM_FILLS = [
    (1, 0), (2, 0), (2, 1), (3, 0), (3, 1),
    (4, 0), (4, 1), (4, 2), (4, 3),
    (5, 0), (5, 1), (5, 2), (5, 3),
]


def build_nc(NT=16):
    N = NT * P
    NP = NT // 2  # tile pairs
    nc = bacc.Bacc()

    x_d = nc.dram_tensor("x", [N, D], F32, kind="ExternalInput")
    x2_d = nc.dram_tensor("x2", [N, D], F32, kind="ExternalInput")
    x3_d = nc.dram_tensor("x3", [N, D], F32, kind="ExternalInput")
    wt1_d = nc.dram_tensor("wt1", [D, D], FP8, kind="ExternalInput")  # 32*f*W1^T
    wt2_d = nc.dram_tensor("wt2", [D, D], FP8, kind="ExternalInput")  # 32*f*W2^T
    fb_d = nc.dram_tensor("fb", [1, D], F32, kind="ExternalInput")  # 8192*f*(b1+b2)
    id_d = nc.dram_tensor("ident", [P, P], F32, kind="ExternalInput")  # np.eye
    out_d = nc.dram_tensor("out", [N, D], BF16, kind="ExternalOutput")

    x2_pr = x2_d.rearrange("(q t p) d -> q p t d", t=2, p=P)   # 2-tile pairs
    x3_ch = x3_d.rearrange("(c t p) d -> c p t d", t=4, p=P)   # 4-tile chunks
    x_ch = x_d.rearrange("(c t p) d -> c p t d", t=4, p=P)
    out_t = out_d.rearrange("(t p) d -> t p d", p=P)

    with tile.TileContext(nc) as tc:
        with (
            tc.tile_pool(name="ps", bufs=1, space="PSUM") as psp,
            tc.tile_pool(name="consts", bufs=1) as consts,
            tc.tile_pool(name="big", bufs=1) as big,
            tc.tile_pool(name="stream", bufs=3) as stream,
            tc.tile_pool(name="stats", bufs=2) as stats,
            tc.tile_pool(name="xtip", bufs=3) as xtip,
            tc.tile_pool(name="outp", bufs=3) as outp,
        ):
            psb = psp.tile([P, 4096], F32)

            # ---- constants ----
            ones8 = consts.tile([P, 2, P], FP8)
            nc.vector.memset(ones8, 1.0)
            onesfb = consts.tile([P, P], BF16)
            nc.vector.memset(onesfb, 1.0 / 128.0)
            ident = consts.tile([P, P], F32)
            ident8 = consts.tile([P, P], FP8)
            fbb = consts.tile([P, D], F32)
            fbby = consts.tile([P, D], BF16)
            wts = consts.tile([P, 2, DT, D], FP8)
            kqt = [
                consts.tile([P, DT, D], FP8, tag=f"kqt{t}", name=f"kqt{t}")
                for t in range(2)
            ]
            msum = consts.tile([P, DT, D], FP8)
            rscol = consts.tile([P, 2, DT], F32)  # (CKQ/C1)/S per d'
            xbig = consts.tile([P, NT, D], F32)
            gate = consts.tile([P, 1], F32)
            g = [
                big.tile([P, NT, D], FP8, tag=f"g{t}", name=f"g{t}")
                for t in range(2)
            ]
            sc = [
                big.tile([P, NT, D], FP8, tag=f"sc{t}", name=f"sc{t}")
                for t in range(2)
            ]

            # ---- merged stream phase: x2 & x3 concurrently ----
            rvec = stats.tile([P, 2, NT], F32, tag="rvec")
            rrec = stats.tile([P, 2, NT], F32, tag="rrec")
            xch = [None, None]
            for q in range(NP):
                for t in range(2):
                    if t == 0:
                        xi = stream.tile([P, 2, D], F32, tag="in2", bufs=3)
                        nc.sync.dma_start(out=xi, in_=x2_pr[q])
                        if q == 0:
                            # keep gpsimd DMAs (weights/consts) off the HBM
                            # until the first input pair has landed
                            nc.gpsimd.dma_start(out=gate, in_=xi[:, 0, 0:1])
                            for tw, wd in enumerate((wt1_d, wt2_d)):
                                nc.gpsimd.dma_start(
                                    out=wts[:, tw],
                                    in_=wd.rearrange("(c p) j -> p c j", p=P),
                                )
                            nc.gpsimd.dma_start(out=ident, in_=id_d[:, :])
                            nc.vector.tensor_copy(ident8, ident)
                            nc.gpsimd.dma_start(
                                out=fbb, in_=fb_d[0:1, :].to_broadcast([P, D])
                            )
                            nc.vector.tensor_copy(fbby, fbb)
                        src = (xi[:, 0, :], xi[:, 1, :])
                    else:
                        if q % 2 == 0:
                            xch[t] = stream.tile([P, 4, D], F32, tag="in3",
                                                 bufs=2, name=f"xch{q}")
                            nc.scalar.dma_start(out=xch[t], in_=x3_ch[q // 2])
                        o = 2 * (q % 2)
                        src = (xch[t][:, o, :], xch[t][:, o + 1, :])
                    for j in range(2):
                        i = 2 * q + j
                        nc.scalar.activation(
                            out=g[t][:, i, :], in_=src[j],
                            func=mybir.ActivationFunctionType.Exp,
                            accum_out=rvec[:, t, i : i + 1],
                        )
                    nc.vector.reciprocal(rrec[:, t, 2 * q : 2 * q + 2],
                                         rvec[:, t, 2 * q : 2 * q + 2])
                    for j in range(2):
                        i = 2 * q + j
                        # sc is SBUF->SBUF: runs on the otherwise-idle gpsimd
                        nc.gpsimd.tensor_scalar(
                            out=sc[t][:, i, :], in0=g[t][:, i, :],
                            scalar1=rrec[:, t, i : i + 1], scalar2=C1,
                            op0=MUL, op1=MUL,
                        )
                    gpr = g[t][:, 2 * q : 2 * q + 2, :]
                    s_ps = psb[:, S_OFF[t] : S_OFF[t] + D]
                    for off, sz in CHUNKS:
                        nc.tensor.matmul(
                            s_ps[:, off : off + sz], ones8,
                            gpr[:, :, off : off + sz],
                            start=(q == 0), stop=(q == NP - 1), perf_mode=DR,
                        )
                    lhsT = sc[t][:, 2 * q : 2 * q + 2, 0:P]
                    d0 = psb[:, DP0_OFF[t] : DP0_OFF[t] + D]
                    for off, sz in CHUNKS:
                        nc.tensor.matmul(
                            d0[:, off : off + sz], lhsT,
                            gpr[:, :, off : off + sz],
                            start=(q == 0), stop=(q == NP - 1), perf_mode=DR,
                        )

            # ---- per-attention epilogue: 1/S, dp0 copy, dp1-5 bursts, fills
            for t in range(2):
                s_ps = psb[:, S_OFF[t] : S_OFF[t] + D]
                rsb = stream.tile([P, D], F32, tag="rsb", bufs=2)
                nc.vector.reciprocal(rsb, s_ps)
                nc.vector.tensor_scalar_mul(rsb, rsb, CKQ / C1)
                ssb = stream.tile([P, D], F32, tag="ssb", bufs=2)
                nc.vector.tensor_scalar_mul(ssb, s_ps, C1 / CKQ)
                for c in range(DT):
                    tp = psb[:, TP_OFF[c % 2] : TP_OFF[c % 2] + P]
                    nc.tensor.transpose(tp, rsb[:, c * P : (c + 1) * P], ident)
                    nc.vector.tensor_copy(rscol[:, t, c : c + 1], tp[:, 0:1])
                # dp0 row copy-out
                nc.vector.tensor_scalar_mul(
                    kqt[t][:, 0, :], psb[:, DP0_OFF[t] : DP0_OFF[t] + D],
                    rscol[:, t, 0:1],
                )
                # dp1-5 bursts (pure PE, g/sc resident)
                for dp in range(1, DT):
                    for off, sz, poff in POST_PLAN[dp]:
                        for q in range(NP):
                            nc.tensor.matmul(
                                psb[:, poff : poff + sz],
                                sc[t][:, 2 * q : 2 * q + 2,
                                      dp * P : (dp + 1) * P],
                                g[t][:, 2 * q : 2 * q + 2, off : off + sz],
                                start=(q == 0), stop=(q == NP - 1),
                                perf_mode=DR,
                            )
                    for off, sz, poff in POST_PLAN[dp]:
                        nc.vector.tensor_scalar_mul(
                            kqt[t][:, dp, off : off + sz],
                            psb[:, poff : poff + sz],
                            rscol[:, t, dp : dp + 1],
                        )
                # lower blocks = transposed upper blocks rescaled.
                # t0 fills run while x3 still streams (ACT busy) -> DVE;
                # t1 fills run post-stream when ACT is free -> ACT cannot do
                # stt, so keep DVE but alternate with scalar-engine copies.
                for fi, (hi, lo) in enumerate(SYM_FILLS):
                    tp = psb[:, TP_OFF[fi % 2] : TP_OFF[fi % 2] + P]
                    nc.tensor.matmul(
                        tp, kqt[t][:, lo, hi * P : (hi + 1) * P], ident8,
                        start=True, stop=True,
                    )
                    nc.vector.scalar_tensor_tensor(
                        out=kqt[t][:, hi, lo * P : (lo + 1) * P],
                        in0=tp, scalar=rscol[:, t, hi : hi + 1],
                        in1=ssb[:, lo * P : (lo + 1) * P],
                        op0=MUL, op1=MUL,
                    )

            # x prefetch on both queues (4-tile chunks)
            for c in range(4):
                eng = nc.sync if c % 2 == 0 else nc.scalar
                eng.dma_start(out=xbig[:, 4 * c : 4 * c + 4, :], in_=x_ch[c])

            # ---- Msum[d, j] = sum_t sum_d' kqt[t][d', d] * wts[t][d', j] ----
            for d in range(DT):
                mb = 1024 * (d % 2)
                m_ps = psb[:, mb : mb + D]
                for t in range(2):
                    for dpp in range(0, DT, 2):
                        lhsT = kqt[t][:, dpp : dpp + 2, d * P : (d + 1) * P]
                        for off, sz in CHUNKS:
                            nc.tensor.matmul(
                                m_ps[:, off : off + sz], lhsT,
                                wts[:, t, dpp : dpp + 2, off : off + sz],
                                start=(t == 0 and dpp == 0),
                                stop=(t == 1 and dpp == DT - 2),
                                perf_mode=DR,
                            )
                if d % 2 == 0:
                    nc.vector.tensor_scalar_mul(msum[:, d, :], m_ps, 1.0 / CM_DIV)
                else:
                    nc.scalar.mul(msum[:, d, :], m_ps, 1.0 / CM_DIV)

            # ---- y = xti @ msum + fb_mm; out = y/8192 + x ----
            for i in range(NT):
                xb = 2048 + 1024 * (i % 2)
                xt_ps = psb[:, xb : xb + D]
                for c in range(DT):
                    nc.tensor.transpose(
                        xt_ps[:, c * P : (c + 1) * P],
                        xbig[:, i, c * P : (c + 1) * P],
                        ident,
                    )
                xti = xtip.tile([P, DT, P], FP8, tag="xti")
                if i % 2 == 0:
                    nc.scalar.mul(xti, xt_ps, CY)
                else:
                    nc.vector.tensor_scalar_mul(xti, xt_ps, CY)
                yb = 1024 * (i % 2)
                y_ps = psb[:, yb : yb + D]
                for off, sz in CHUNKS:
                    nc.tensor.matmul(
                        y_ps[:, off : off + sz], onesfb, fbby[:, off : off + sz],
                        start=True, stop=False,
                    )
                for k in range(0, DT, 2):
                    for off, sz in CHUNKS:
                        nc.tensor.matmul(
                            y_ps[:, off : off + sz],
                            xti[:, k : k + 2, :],
                            msum[:, k : k + 2, off : off + sz],
                            start=False, stop=(k == DT - 2),
                            perf_mode=DR,
                        )
                oi = outp.tile([P, D], F32, tag="out")
                nc.vector.scalar_tensor_tensor(
                    out=oi, in0=y_ps, scalar=Y_SCALE, in1=xbig[:, i, :],
                    op0=MUL, op1=mybir.AluOpType.add,
                )
                nc.gpsimd.dma_start(out=out_t[i], in_=oi)

    nc.compile()
    return nc


def prep_inputs(inputs):
    x = np.ascontiguousarray(np.asarray(inputs["x"], dtype=np.float32))
    x2 = np.ascontiguousarray(np.asarray(inputs["x2"], dtype=np.float32))
    x3 = np.ascontiguousarray(np.asarray(inputs["x3"], dtype=np.float32))
    W1 = np.asarray(inputs["W1"], dtype=np.float32)
    b1 = np.asarray(inputs["b1"], dtype=np.float32)
    W2 = np.asarray(inputs["W2"], dtype=np.float32)
    b2 = np.asarray(inputs["b2"], dtype=np.float32)
    w = np.asarray(inputs["w"], dtype=np.float32)

    f = 1.0 / (1.0 + np.exp(-float(w.reshape(-1)[0])))
    wt1 = np.ascontiguousarray((CW * f * W1).T).astype(ml_dtypes.float8_e4m3fn)
    wt2 = np.ascontiguousarray((CW * f * W2).T).astype(ml_dtypes.float8_e4m3fn)
    fb = (f * (b1 + b2) / Y_SCALE).astype(np.float32).reshape(1, D)

    ident = np.eye(P, dtype=np.float32)
    return [
        {
            "x": x[b], "x2": x2[b], "x3": x3[b],
            "wt1": wt1, "wt2": wt2, "fb": fb, "ident": ident,
        }
        for b in range(B)
    ]


_NC = None


def kernel(**inputs) -> np.ndarray:
    global _NC
    if _NC is None:
        _NC = build_nc()
    in_maps = prep_inputs(inputs)
    res = run_bass_kernel_spmd(_NC, in_maps, list(range(B)))
    return np.stack(
        [res.results[b]["out"].astype(np.float32) for b in range(B)], axis=0
    )
